# revision 46
# baseline (speedup 1.0000x reference)
# Binary (sign) matmul: out[b,m,n] = sum_k sign(x[b,m,k]) * sign(y[b,n,k]) * x_clip * y_clip
# B=2, M=N=K=4096, fp32 in / fp16 out (host upcasts to fp32).
#
# Sharding: 8 cores = batch(2) x 2x2 grid over (M, N). Each core computes a
# [2048, 2048] output block from x[b, mh*2048:, :] and y[b, nh*2048:, :].
# The host binds each core's shards in k-major (transposed) layout — pure
# input marshalling; all arithmetic (sign, matmul, clip scaling) runs on
# device.
#
# Per-core device pipeline:
#   DMA fp32 k-major chunks -> ScalarE Sign (fp32 -> fp8e4 +-1) -> TensorE
#   fp8 matmuls (exact +-1 sums in fp32 PSUM) -> DVE fused spill/close
#   (scalar_tensor_tensor folds the clip product) -> DMA fp16 out.
#
# Schedule: K split in four phases [4,4,4,4] DR-steps (kd = 256 k-values).
#   q0 (kd 0-3):   all 16 i-tiles march; i0,i1's DR matmuls chase the
#                  stream (each waits only its slab slice), i2+ start as
#                  the early spills free psum regions. Spill =
#                  tensor_scalar mult by clip -> fp16 ACC.
#   q1,q2:         DR march all 16 i-tiles, spill = scalar_tensor_tensor
#                  (psum*clip + ACC) -> ACC, one fused op per 2-bank tile.
#   q3 (kd 12-15): DR march, close = scalar_tensor_tensor
#                  (psum*clip + ACC) -> fp16 OT, DMA out. A 4-kd final
#                  phase starts after the stream tail has landed, so no
#                  march ever holds psum open waiting on a DMA landing.
#
# PSUM: per i-tile two [P,1024] tiles (2 banks / 2 n-chunks each) from a
# 4-buffer pool (all 8 banks). Matmuls are emitted kd-outer so consecutive
# MMs share the stationary operand and _dedupe_ldweights collapses their
# LDWEIGHTS (steady-state DR matmul issue gap: 216 ns = 512 cycles at
# 2.4 GHz — the streaming roofline). The fused spill frees a whole tile at
# once, keeping the next i-tile's chains kd-outer too.
#
# DMA staging is half-chunk ([P,1024] fp32) with a 9-deep pool: the
# stage-buffer pipeline is latency-limited (~12us per chunk round trip),
# so depth sets the stream rate; 9 halves ~ 4.5 chunks in flight keeps the
# DMA at its ~400 GB/s ceiling.
#
# PSUM: tiles are [P, 1024] (2 banks, 2 n-chunks each), 4-buffer pool = all
# 8 banks. A single fused spill frees both banks of a tile at once, so the
# Tile scheduler sees all of an i-tile's chains become ready together and
# emits the matmuls kd-outer — consecutive MMs then share the stationary
# operand and _dedupe_ldweights collapses their LDWEIGHTS.
#
# SBUF: fp16 ACC is 64KB/partition; sign tensors held as per-phase slabs
# (8 k-chunks = 16KB each) in 3-buffer pools: slab 3 reuses slab 0's memory
# after the q0 march (program order puts the reads first for WAR tracking).
import numpy as np

B = 2
M = N = K = 4096
P = 128
MSH, NSH = 2048, 2048      # per-core shard of M, N
KO = K // P                # 32 k-chunks of 128
MT = MSH // P              # 16 m row-tiles
FD = 512                   # matmul free dim
NCH = NSH // FD            # 4 n chunks
NCORES = 8

KD = KO // 2               # 16 DoubleRow k-steps of 256
SLAB = 8                   # k-chunks per slab (4 DR steps)
NSLAB = KO // SLAB         # 4 slabs: q0, q1, Ha, Hb


def _build_program():
    import concourse.bacc as bacc
    import concourse.mybir as mybir
    import concourse.tile as tile
    from concourse.bass import ts

    f32 = mybir.dt.float32
    f16 = mybir.dt.float16
    fp8 = mybir.dt.float8e4
    Sign = mybir.ActivationFunctionType.Sign
    Add = mybir.AluOpType.add
    Mult = mybir.AluOpType.mult

    nc = bacc.Bacc(
        "TRN2",
        target_bir_lowering=False,
        debug=False,
        num_devices=NCORES,
    )
    xsT = nc.dram_tensor("xsT", [K, MSH], f32, kind="ExternalInput").ap()
    ysT = nc.dram_tensor("ysT", [K, NSH], f32, kind="ExternalInput").ap()
    clips = nc.dram_tensor("clips", [P, 2], f32, kind="ExternalInput").ap()
    out = nc.dram_tensor("out", [MSH, NSH], f16, kind="ExternalOutput").ap()

    with tile.TileContext(nc) as tc:
        with (
            tc.tile_pool(name="constp", bufs=1) as constp,
            tc.tile_pool(name="sxp", bufs=3) as sxp,
            tc.tile_pool(name="syp", bufs=3) as syp,
            tc.tile_pool(name="accp", bufs=1) as accp,
            tc.tile_pool(name="stagep", bufs=9) as stagep,
            tc.tile_pool(name="outp", bufs=2) as outp,
            tc.tile_pool(name="psump", bufs=4, space="PSUM") as psump,
        ):
            # fp16 accumulator, clip-scaled partial sums: [P, MT, 2048]
            ACC = accp.tile([P, MT, NSH], f16, name="ACC")

            HF = MSH // 2

            def prep_half(src_dram, ko, dst, kol, half, pieces=1):
                """DMA+sign one half-chunk; `pieces` subdivides it further
                (used for the first chunks so the chase starts ASAP)."""
                st = stagep.tile([P, HF], f32, name="st", tag="stage")
                base = half * HF
                w = HF // pieces
                for q in range(pieces):
                    nc.sync.dma_start(
                        st[:, q * w : (q + 1) * w],
                        src_dram[ts(ko, P), base + q * w : base + (q + 1) * w],
                    )
                    nc.scalar.activation(
                        dst[:, kol, base + q * w : base + (q + 1) * w],
                        st[:, q * w : (q + 1) * w],
                        Sign,
                    )

            def prep(src_dram, ko, dst, kol):
                for half in (0, 1):
                    prep_half(src_dram, ko, dst, kol, half)

            # Sign slabs, filled in stream order (x and y interleaved per ko).
            # Slab s covers k-chunks [s*SLAB, (s+1)*SLAB). With 3-buffer
            # pools, slab 3 reuses slab 0's memory; its preps are emitted
            # after the q0 march (the last reader of slab 0) so the pool's
            # WAR tracking sees the reads first in program order.
            xslabs, yslabs = [], []

            def emit_slab(s):
                sx = sxp.tile([P, SLAB, MSH], fp8, name=f"sx{s}", tag="sx")
                sy = syp.tile([P, SLAB, NSH], fp8, name=f"sy{s}", tag="sy")
                xslabs.append(sx)
                yslabs.append(sy)
                for kol in range(SLAB):
                    ko = s * SLAB + kol
                    prep(xsT, ko, sx, kol)
                    prep(ysT, ko, sy, kol)

            emit_slab(0)

            # clip product, replicated per-partition: [P, 1] — emitted
            # after slab 0's preps so the first input DMAs lead the queue.
            clip_sb = constp.tile([P, 2], f32)
            nc.sync.dma_start(clip_sb[:], clips)
            clip_prod = constp.tile([P, 1], f32)
            nc.vector.tensor_tensor(
                clip_prod[:], clip_sb[:, 0:1], clip_sb[:, 1:2],
                mybir.AluOpType.mult,
            )

            emit_slab(1)
            emit_slab(2)

            def dr_mm(ps_ap, s, kdl, i, nch, first, last):
                nc.tensor.matmul(
                    ps_ap,
                    lhsT=xslabs[s][:, 2 * kdl : 2 * kdl + 2, ts(i, P)],
                    rhs=yslabs[s][:, 2 * kdl : 2 * kdl + 2, ts(nch, FD)],
                    start=first,
                    stop=last,
                    perf_mode=mybir.MatmulPerfMode.DoubleRow,
                )

            def march(i, kds, spill, chunk_head=False):
                """One i-tile: DR-accumulate over kds into 2 two-bank psum
                tiles (kd-outer emission so consecutive MMs share lhsT),
                then one fused spill per tile. chunk_head splits the first
                DR step into two normal-mode single-chunk MMs so the chase
                tiles start ~10us earlier (one chunk landed vs two)."""
                pa = psump.tile([P, 2 * FD], f32, name="pa", tag="ps")
                pb = psump.tile([P, 2 * FD], f32, name="pb", tag="ps")
                last = len(kds) - 1
                for j, (s, kdl) in enumerate(kds):
                    if j == 0 and chunk_head:
                        for c in (0, 1):
                            for nch in range(NCH):
                                tile_ap = (
                                    pa if nch < 2 else pb
                                )[:, ts(nch % 2, FD)]
                                nc.tensor.matmul(
                                    tile_ap,
                                    lhsT=xslabs[s][:, 2 * kdl + c, ts(i, P)],
                                    rhs=yslabs[s][:, 2 * kdl + c, ts(nch, FD)],
                                    start=(c == 0),
                                    stop=False,
                                )
                        continue
                    for nch in range(NCH):
                        tile_ap = (pa if nch < 2 else pb)[:, ts(nch % 2, FD)]
                        dr_mm(tile_ap, s, kdl, i, nch,
                              j == 0 and not chunk_head, j == last)
                spill(i, 0, pa)
                spill(i, 1, pb)

            # q0 spill: plain clip-scaled copy; one op per 2-bank tile.
            def spill_q0(i, h, ps):
                nc.vector.tensor_scalar_mul(
                    ACC[:, i, h * 2 * FD : (h + 1) * 2 * FD],
                    ps[:],
                    clip_prod[:],
                )

            def spill_add(i, h, ps):
                a = ACC[:, i, h * 2 * FD : (h + 1) * 2 * FD]
                nc.vector.scalar_tensor_tensor(
                    a, ps[:], clip_prod[:], a, Mult, Add
                )

            # Four 4-kd phases; kd k lives in slab k//4 at local index k%4.
            def kds(lo, hi):
                return [(k // 4, k % 4) for k in range(lo, hi)]

            Q0, Q1, Q2, Q3 = kds(0, 4), kds(4, 8), kds(8, 12), kds(12, 16)

            # ---- q0 march: i0,i1 naturally CHASE the stream (their DR
            # MMs wait on each slab slice as it lands and psum holds all
            # four of their tiles); i2+ start as the early spills free
            # psum regions — i.e. once slab 0 has fully landed. ----
            for i in range(MT):
                march(i, Q0, spill_q0, chunk_head=(i < 2))

            # slab 3 (ko 24-31) reuses slab 0's buffers — emit after q0,
            # slab 0's last reader.
            emit_slab(3)

            # ---- q1, q2: fused spill ACC = psum*clip + ACC ----
            # (4-kd phases: no accumulation group ever straddles the
            # stream tail, so marches never hold psum regions open waiting
            # on a DMA landing.)
            for i in range(MT):
                march(i, Q1, spill_add)
            for i in range(MT):
                march(i, Q2, spill_add)

            # ---- q3: kd 12..15, close: OT = psum*clip + ACC, DMA out ----
            def close_h(ot):
                def fn(i, h, ps):
                    o = ot[:, h * 2 * FD : (h + 1) * 2 * FD]
                    nc.vector.scalar_tensor_tensor(
                        o, ps[:], clip_prod[:],
                        ACC[:, i, h * 2 * FD : (h + 1) * 2 * FD],
                        Mult, Add,
                    )
                return fn

            for i in range(MT):
                ot = outp.tile([P, NSH], f16, name="ot")
                march(i, Q3, close_h(ot))
                nc.sync.dma_start(out[ts(i, P), :], ot[:])

    nc.compile()
    removed = _dedupe_ldweights(nc)
    import os
    if os.environ.get("KERNEL_DEBUG"):
        print(f"dedupe_ldweights removed {removed}")
    return nc


def _dedupe_ldweights(nc):
    """Drop redundant standalone InstLdweights left by bacc's matmul split.

    Consecutive matmuls sharing one stationary tile still get one
    InstLdweights each; an InstLdweights identical to the previous one
    (same AP, same mode) with no semaphore waits/updates is a no-op."""
    removed = 0
    for blk in nc.m.functions[0].blocks:
        prev_key = None
        keep = []
        for inst in blk.instructions:
            nm = type(inst).__name__
            if nm == "InstLdweights":
                pap = inst.ins[0]
                key = (
                    pap.memref,
                    pap.offset,
                    str(pap.ap),
                    str(pap.dtype),
                    str(inst.perf_mode),
                    str(inst.is_transpose),
                )
                if (
                    key == prev_key
                    and not inst.has_wait()
                    and not inst.has_update()
                ):
                    removed += 1
                    continue
                prev_key = key
            keep.append(inst)
        if removed:
            blk.instructions = keep
    return removed


_PROGRAM_CACHE = None


def _get_program():
    global _PROGRAM_CACHE
    if _PROGRAM_CACHE is None:
        _PROGRAM_CACHE = _build_program()
    return _PROGRAM_CACHE


def _shard_inputs(x, y, x_clip, y_clip):
    x = np.asarray(x, dtype=np.float32)
    y = np.asarray(y, dtype=np.float32)
    clips = np.empty((P, 2), dtype=np.float32)
    clips[:, 0] = np.float32(x_clip)
    clips[:, 1] = np.float32(y_clip)
    in_maps = []
    for c in range(NCORES):
        b, mh, nh = c // 4, (c % 4) // 2, c % 2
        in_maps.append(
            {
                "xsT": np.ascontiguousarray(x[b, mh * MSH : (mh + 1) * MSH, :].T),
                "ysT": np.ascontiguousarray(y[b, nh * NSH : (nh + 1) * NSH, :].T),
                "clips": clips,
            }
        )
    return in_maps


def run_sharded(x, y, x_clip, y_clip, trace=False, **kwargs):
    """Run the SPMD kernel; returns (out, BassKernelResults)."""
    from concourse.bass_utils import run_bass_kernel_spmd

    nc = _get_program()
    in_maps = _shard_inputs(x, y, x_clip, y_clip)
    res = run_bass_kernel_spmd(
        nc, in_maps, core_ids=list(range(NCORES)), trace=trace, **kwargs
    )
    out = np.empty((B, M, N), dtype=np.float32)
    for c in range(NCORES):
        b, mh, nh = c // 4, (c % 4) // 2, c % 2
        out[b, mh * MSH : (mh + 1) * MSH, nh * NSH : (nh + 1) * NSH] = (
            res.results[c]["out"].astype(np.float32)
        )
    return out, res


def kernel(x, y, x_clip, y_clip):
    out, _ = run_sharded(x, y, x_clip, y_clip, trace=False)
    return out
